# revision 28
# baseline (speedup 1.0000x reference)
"""Trainium2 Bass kernel for an attention block (pre-LN attn + pre-LN SiLU MLP).

Sharding: data-parallel over batch b — one batch element per NeuronCore, 8 cores,
no collectives. Each core runs the full block on its [4096, 256] slice.

v4 design (per core, T=4096 tokens, d=256, mlp=1024), evolved from v3 by
HW ablation profiling (per-phase/per-op skip builds timed on trn2):
  LN applies are BATCHED: one subtract + one multiply tensor_tensor over a
      whole 8-block group with per-block mean/rstd broadcast via stride-0
      APs (DVE sub + Pool mult), replacing 2 gpsimd tensor_scalar ops per
      block pair (GPSIMD ~2.6 cyc/elem made the per-block applies the
      kernel's longest critical path: ~75us LN1 + ~92us LN2 on HW).
  Transposes are PACKED: adjacent bf16 feature pairs are bitcast to f32 and
      transposed as ONE PE f32 transpose per 128x256 block (halves the
      transpose instruction count; LDWEIGHTS per matmul is ~40-120ns on HW
      and unmodeled in CoreSim). The PSUM result is unpacked by the
      ACT/DVE copy (strided bf16 read -> fp8 xnT write), so the feature
      order inside the 256-contraction becomes f = 2p + j; the host
      permutes the contraction rows of wq/wk/wv/w1 to match (tr_pack).
  Q/K/V projections: fp8 DoubleRow single-shot matmuls (16x-scaled fp8
      weights) -> f32 PSUM -> fp8 QT/KT (ACT Identity + bias) and V (DVE).
  S' = K8 @ Q8^T fp8 DoubleRow, [P,1024] PSUM chunks; A8 = exp(S'/4096) as
      fp8e4m3 alternating DVE (Schraudolph u8 bit-trick) / ACT (true Exp).
  O[i, :258] = A8 @ V8 fp8 DoubleRow with a 16.0 tail column in V giving
      row sums; epilogue x2 = x + O/rowsum (ACT scale-copy + Pool add).
  MLP: TG=256 token groups (batched LN2 apply -> packed transpose -> W1
      matmul + fused SiLU+bias on ACT -> W2 matmul -> DVE residual epi);
      small groups pipeline the LN2->transpose->matmul chain much better
      than 1024-wide groups (-30us on HW).
  Timing builds use a 2x-unrolled For_i body (loop_unroll=2): halves the
      per-iteration all-engine barrier count and lets adjacent reps overlap.
  v5: attention i-groups shrunk to IG=512 with PAIRED S tiles (conv_pair):
      each softmax conversion covers two j-chunks ([P,2,512] PSUM) in one
      ACT/DVE op, halving conversion dispatch count; the smaller AT tile
      (16KB/buf) allows at_bufs=3 for deeper exp/AV overlap (-17us on HW).
Measured: ~304,000 ns / rep on 8 cores (slope method), rel_l2 4.37e-3
(gate 2e-2). v3 baseline was 487,718 ns.
"""

import numpy as np

import concourse.bacc as bacc
import concourse.mybir as mybir
import concourse.tile as tile
from concourse.bass_utils import run_bass_kernel_spmd
from concourse.masks import make_identity

P = 128
D = 256
KD = D // P            # 2 d-chunks of 128
M = 1024
MO = M // P            # 8 mlp chunks of 128
EPS = 1e-5
WS = 16.0              # fp8 weight prescale (q,k,v all carry 16x)
SCALE = 1.0 / 16.0     # d ** -0.5
SCALE_S = SCALE / (WS * WS)             # logits = S' * SCALE_S
A8 = SCALE_S * 8.0 / float(np.log(2.0))  # S' -> e4m3 exponent-code scale
B8 = 56.0 + 0.5 - 0.344                 # 7*8 bias + trunc-centering - ln-corr

F32 = mybir.dt.float32
BF16 = mybir.dt.bfloat16
F8 = mybir.dt.float8e4
U8 = mybir.dt.uint8
AF = mybir.ActivationFunctionType
ALU = mybir.AluOpType

N_CORES = 8
FULL_T = 4096


def build(T=FULL_T, n_cores=N_CORES, silu_af=None, reps=1, loop_reps=None,
          at_bufs=2, sps_bufs=3, ops_bufs=2, qkq_eng="mix",
          v_eng="dve", zb=False, skip_attn=False, skip_mlp=False,
          stats_mode="bn", epi_mode="dve", attn_loop=False, mlp_loop=False,
          dma_tr=False, hps_bufs=2, qk_bufs=2, defer_rstd=False,
          tweak=0, cp_eng="mix", ln2_late=False,
          conv_engines=("act", "dve"), ablate=(), ln_batch="off",
          tr_pack=False, loop_unroll=1, x_chunks=(4, 4, 8, 16),
          ig_width=1024, tg_width=1024, conv_pair=False,
          rsqrt=False, cc_width=1024, out_merge=False):
    silu_af = silu_af or AF.Silu
    VW = 272               # V row width; fp8 pair-step must be 16B-aligned
    NB = T // P            # token blocks
    JC = NB                # j-chunks (keys)
    IG = min(ig_width, T)  # i-group width
    NIG = T // IG
    IB = IG // P           # i-blocks per group
    TG = min(tg_width, T)  # mlp token-group width
    NTG = T // TG
    TB = TG // P
    CC = min(cc_width, T)  # qk projection column chunk
    NCC = T // CC
    G8 = min(8, NB)        # LN stats group

    nc = bacc.Bacc("TRN2", target_bir_lowering=False, debug=False,
                   num_devices=n_cores)

    x_d = nc.dram_tensor("x", [T, D], F32, kind="ExternalInput")
    wqT_d = nc.dram_tensor("wqT8", [D, D], F8, kind="ExternalInput")
    wkT_d = nc.dram_tensor("wkT8", [D, D], F8, kind="ExternalInput")
    wvT_d = nc.dram_tensor("wvT8", [D, D], F8, kind="ExternalInput")
    rq_d = nc.dram_tensor("rq16", [D], F32, kind="ExternalInput")
    rk_d = nc.dram_tensor("rk16", [D], F32, kind="ExternalInput")
    rv_d = nc.dram_tensor("rv16", [D], F32, kind="ExternalInput")
    w1T_d = nc.dram_tensor("w1T", [D, M], BF16, kind="ExternalInput")
    b1r_d = nc.dram_tensor("b1r", [M], F32, kind="ExternalInput")
    w2T_d = nc.dram_tensor("w2T", [M, D], BF16, kind="ExternalInput")
    b2_d = nc.dram_tensor("b2", [D], F32, kind="ExternalInput")
    onesv_d = nc.dram_tensor("onesv16_f8", [VW - D], F8, kind="ExternalInput")
    out_d = nc.dram_tensor("out", [T, D], F32, kind="ExternalOutput")

    import contextlib

    with tile.TileContext(nc) as tc:
      if loop_reps is not None:
          assert loop_reps % loop_unroll == 0
          rep_iter = range(loop_unroll)
          loop_cm = tc.For_i(0, loop_reps // loop_unroll, 1)
      else:
          rep_iter = range(reps)
          loop_cm = contextlib.nullcontext()
      with loop_cm:
       for _rep in rep_iter:
        with tc.tile_pool(name="glob", bufs=1) as glob:
            xsb = glob.tile([P, NB, D], F32)
            x_r = x_d.ap().rearrange("(p tt) d -> p tt d", p=P)
            c0 = 0
            for xq in tuple(x_chunks) + (NB,):
                xq = min(xq, NB - c0)
                if xq <= 0:
                    break
                nc.sync.dma_start(xsb[:, c0:c0 + xq, :], x_r[:, c0:c0 + xq, :])
                c0 += xq
            if not zb:
                b2b = glob.tile([P, D], F32)
                nc.sync.dma_start(b2b[:],
                                  b2_d.ap()[None, :].to_broadcast([P, D]))
                rvb = glob.tile([P, D], F32)
                nc.sync.dma_start(rvb[:],
                                  rv_d.ap()[None, :].to_broadcast([P, D]))
            rqs = glob.tile([P, KD], F32)
            nc.sync.dma_start(rqs[:], rq_d.ap().rearrange("(ko p) -> p ko", p=P))
            rks = glob.tile([P, KD], F32)
            nc.sync.dma_start(rks[:], rk_d.ap().rearrange("(ko p) -> p ko", p=P))
            b1rs = glob.tile([P, MO], F32)
            nc.sync.dma_start(b1rs[:], b1r_d.ap().rearrange("(mo p) -> p mo", p=P))
            identb = glob.tile([P, P], BF16)
            make_identity(nc, identb)
            if tr_pack:
                identf = glob.tile([P, P], F32)
                make_identity(nc, identf)
            epst = glob.tile([P, 1], F32)
            nc.vector.memset(epst[:], EPS)
            for _tw in range(tweak):
                nc.vector.memset(epst[:], EPS)
            mv2 = glob.tile([P, NB, 2], F32)
            xn2T_all = glob.tile([P, 2, KD, TG], BF16)
            w1s = glob.tile([P, KD, M], BF16)
            nc.sync.dma_start(w1s[:], w1T_d.ap().rearrange(
                "(ko p) m -> p ko m", p=P))
            w2s = glob.tile([P, MO, D], BF16)
            nc.sync.dma_start(w2s[:], w2T_d.ap().rearrange(
                "(mo p) m -> p mo m", p=P))

            def ln_apply(dst, src, mv_all, b, eng=None):
                eng = eng or nc.gpsimd
                eng.tensor_scalar(out=dst, in0=src,
                                  scalar1=mv_all[:, b, 0:1],
                                  scalar2=mv_all[:, b, 1:2],
                                  op0=ALU.subtract, op1=ALU.mult)

            def ln_apply_group(dst, g, gn, mv_all, tmp_pool):
                """Batched (x - mu) * rstd for gn blocks in one sub + one
                mult, per-block scalars broadcast along D (stride-0 AP).
                Engines per ln_batch: 'pd' = Pool sub + DVE mult, 'dp' =
                DVE sub + Pool mult."""
                tmp8 = tmp_pool.tile([P, gn, D], F32, tag="lb_tmp")
                e0, e1 = ((nc.gpsimd, nc.vector) if ln_batch == "pd"
                          else (nc.vector, nc.gpsimd))
                e0.tensor_tensor(
                    out=tmp8[:], in0=xsb[:, g:g + gn, :],
                    in1=mv_all[:, g:g + gn, 0:1].to_broadcast([P, gn, D]),
                    op=ALU.subtract)
                e1.tensor_tensor(
                    out=dst, in0=tmp8[:],
                    in1=mv_all[:, g:g + gn, 1:2].to_broadcast([P, gn, D]),
                    op=ALU.mult)

            def ln_group_stats(mv_all, g, gn, xsq_pool, fin_pool):
                """Batched stats for blocks [g, g+gn): Pool squares + DVE
                reduces + fused finalize -> mean in [...,0], rstd in [...,1]."""
                if stats_mode == "acc" and mv_all is not mv2:
                    # LN1 stats via ACT accumulate (DVE off the chain head)
                    for bs in range(g, g + gn):
                        scr = xsq_pool.tile([P, D], F32, tag="ascr")
                        nc.scalar.activation(scr[:], xsb[:, bs, :], AF.Copy,
                                             accum_out=mv_all[:, bs, 0:1])
                        scr2 = xsq_pool.tile([P, D], F32, tag="ascr2")
                        nc.scalar.activation(scr2[:], xsb[:, bs, :],
                                             AF.Square,
                                             accum_out=mv_all[:, bs, 1:2])
                    nc.vector.tensor_scalar_mul(out=mv_all[:, g:g + gn, 0],
                                                in0=mv_all[:, g:g + gn, 0],
                                                scalar1=1.0 / D)
                    musq = fin_pool.tile([P, gn], F32, tag="amusq")
                    nc.vector.tensor_tensor(out=musq[:],
                                            in0=mv_all[:, g:g + gn, 0],
                                            in1=mv_all[:, g:g + gn, 0],
                                            op=ALU.mult)
                    nc.vector.scalar_tensor_tensor(
                        out=mv_all[:, g:g + gn, 1],
                        in0=mv_all[:, g:g + gn, 1], scalar=1.0 / D,
                        in1=musq[:], op0=ALU.mult, op1=ALU.subtract)
                    nc.scalar.activation(mv_all[:, g:g + gn, 1],
                                         mv_all[:, g:g + gn, 1],
                                         AF.Sqrt, bias=epst[:], scale=1.0)
                    nc.vector.reciprocal(mv_all[:, g:g + gn, 1],
                                         mv_all[:, g:g + gn, 1])
                    return
                if stats_mode in ("bn", "acc"):
                    for bs in range(g, g + gn):
                        stats = xsq_pool.tile([P, 6], F32, tag="bst")
                        nc.vector.bn_stats(stats[:], xsb[:, bs, :])
                        nc.vector.bn_aggr(mv_all[:, bs, :], stats[:])
                    if defer_rstd and mv_all is mv2:
                        return   # sqrt+recip batched before the MLP
                    if rsqrt:
                        nc.scalar.activation(mv_all[:, g:g + gn, 1],
                                             mv_all[:, g:g + gn, 1],
                                             AF.Rsqrt, bias=epst[:], scale=1.0)
                        return
                    nc.scalar.activation(mv_all[:, g:g + gn, 1],
                                         mv_all[:, g:g + gn, 1],
                                         AF.Sqrt, bias=epst[:], scale=1.0)
                    nc.vector.reciprocal(mv_all[:, g:g + gn, 1],
                                         mv_all[:, g:g + gn, 1])
                    return
                xsq = xsq_pool.tile([P, gn, D], F32, tag="xsq")
                nc.gpsimd.tensor_tensor(out=xsq[:], in0=xsb[:, g:g + gn, :],
                                        in1=xsb[:, g:g + gn, :], op=ALU.mult)
                nc.vector.tensor_reduce(out=mv_all[:, g:g + gn, 0],
                                        in_=xsb[:, g:g + gn, :],
                                        axis=mybir.AxisListType.X, op=ALU.add)
                nc.vector.tensor_reduce(out=mv_all[:, g:g + gn, 1],
                                        in_=xsq[:],
                                        axis=mybir.AxisListType.X, op=ALU.add)
                nc.vector.tensor_scalar_mul(out=mv_all[:, g:g + gn, 0],
                                            in0=mv_all[:, g:g + gn, 0],
                                            scalar1=1.0 / D)
                musq = fin_pool.tile([P, gn], F32, tag="musq")
                nc.vector.tensor_tensor(out=musq[:], in0=mv_all[:, g:g + gn, 0],
                                        in1=mv_all[:, g:g + gn, 0],
                                        op=ALU.mult)
                nc.vector.scalar_tensor_tensor(
                    out=mv_all[:, g:g + gn, 1], in0=mv_all[:, g:g + gn, 1],
                    scalar=1.0 / D, in1=musq[:], op0=ALU.mult,
                    op1=ALU.subtract)
                nc.scalar.activation(mv_all[:, g:g + gn, 1],
                                     mv_all[:, g:g + gn, 1],
                                     AF.Sqrt, bias=epst[:], scale=1.0)
                nc.vector.reciprocal(mv_all[:, g:g + gn, 1],
                                     mv_all[:, g:g + gn, 1])

            # ---- span: tensors alive through phase A + attention ----
            with tc.tile_pool(name="span", bufs=1) as span:
                QT = span.tile([P, KD, T], F8)
                KT = span.tile([P, KD, T], F8)
                V = span.tile([P, NB, VW], F8)
                nc.sync.dma_start(
                    V[:, :, D:VW],
                    onesv_d.ap()[None, None, :].to_broadcast([P, NB, VW - D]))

                # ---- phase A: LN1, xnT, QKV projections (fp8) ----
                with tc.tile_pool(name="pa", bufs=1) as pa, \
                     tc.tile_pool(name="pa_tmp", bufs=4) as pat, \
                     tc.tile_pool(name="pa_tr", bufs=2, space="PSUM") as paps, \
                     tc.tile_pool(name="pa_v", bufs=2, space="PSUM") as paps2, \
                     tc.tile_pool(name="pa_qk", bufs=qk_bufs, space="PSUM") as paps3:
                    xnT = pa.tile([P, KD, T], F8)
                    wqs = pa.tile([P, KD, D], F8)
                    nc.sync.dma_start(wqs[:], wqT_d.ap().rearrange(
                        "(ko p) m -> p ko m", p=P))
                    wks = pa.tile([P, KD, D], F8)
                    nc.sync.dma_start(wks[:], wkT_d.ap().rearrange(
                        "(ko p) m -> p ko m", p=P))
                    wvs = pa.tile([P, KD, D], F8)
                    nc.sync.dma_start(wvs[:], wvT_d.ap().rearrange(
                        "(ko p) m -> p ko m", p=P))

                    mv1 = pa.tile([P, NB, 2], F32)
                    if "stats" in ablate:
                        nc.vector.memset(mv1[:], 1.0)
                    qki = 0
                    for g in range(0, NB, G8):
                        if "stats" not in ablate:
                            ln_group_stats(mv1, g, G8, pat, pat)
                        if ln_batch != "off" and "lnap" not in ablate:
                            xt8 = pat.tile([P, G8, D], BF16, tag="xn8")
                            ln_apply_group(xt8[:], g, G8, mv1, pat)
                        for b in range(g, g + G8, 2):
                            if ln_batch != "off" and "lnap" not in ablate:
                                xt = xt8[:, b - g:b - g + 2, :]
                            else:
                                xt = pat.tile([P, 2, D], BF16, tag="xn")
                                if "lnap" not in ablate:
                                    ln_apply(xt[:, 0, :], xsb[:, b, :], mv1, b)
                                    ln_apply(xt[:, 1, :], xsb[:, b + 1, :],
                                             mv1, b + 1)
                                else:
                                    nc.gpsimd.tensor_copy(xt[:],
                                                          xsb[:, b:b + 2, :])
                            if "tr" in ablate:
                                if b == 0:
                                    nc.scalar.activation(
                                        xnT[:, :, 0:2 * P]
                                        .rearrange("p k (b q) -> p b k q", b=2),
                                        xt[:], AF.Copy)
                            elif tr_pack:
                                tpsF = paps.tile([P, 2, P], F32, tag="trf")
                                for bi in range(2):
                                    nc.tensor.transpose(
                                        tpsF[:, bi, :],
                                        xt[:, bi, :].bitcast(F32), identf[:])
                                srcb = tpsF[:].bitcast(BF16).rearrange(
                                    "p b (t j) -> p b j t", j=2)
                                dstb = xnT[:, :, b * P:(b + 2) * P].rearrange(
                                    "p j (b q) -> p b j q", b=2)
                                if cp_eng == "mix" and (b // 2) % 2 == 0:
                                    nc.vector.tensor_copy(dstb, srcb)
                                else:
                                    nc.scalar.activation(dstb, srcb, AF.Copy)
                            elif dma_tr:
                                tps = pat.tile([P, 2, KD, P], BF16, tag="xtb")
                                for bi in range(2):
                                    for k in range(KD):
                                        nc.sync.dma_start_transpose(
                                            tps[:, bi, k, :],
                                            xt[:, bi, k * P:(k + 1) * P])
                            else:
                                tps = paps.tile([P, 2, KD, P], BF16, tag="tr")
                                for bi in range(2):
                                    for k in range(KD):
                                        nc.tensor.transpose(
                                            tps[:, bi, k, :],
                                            xt[:, bi, k * P:(k + 1) * P],
                                            identb[:])
                            if "tr" not in ablate and not tr_pack:
                                if cp_eng == "mix" and (b // 2) % 2 == 0:
                                    nc.vector.tensor_copy(
                                        xnT[:, :, b * P:(b + 2) * P]
                                        .rearrange("p k (b q) -> p b k q", b=2),
                                        tps[:])
                                else:
                                    nc.scalar.activation(
                                        xnT[:, :, b * P:(b + 2) * P]
                                        .rearrange("p k (b q) -> p b k q", b=2),
                                        tps[:], AF.Copy)
                            if "v" not in ablate:
                                vps = paps2.tile([P, 2, D], F32, tag="v")
                                for bi in range(2):
                                    nc.tensor.matmul(
                                        vps[:, bi, :],
                                        xnT[:, :, (b + bi) * P:(b + bi + 1) * P],
                                        wvs[:], start=True, stop=True,
                                        perf_mode=mybir.MatmulPerfMode.DoubleRow)
                                if v_eng == "dvecopy":
                                    nc.vector.tensor_copy(V[:, b:b + 2, 0:D],
                                                          vps[:])
                                elif zb or v_eng == "act":
                                    nc.scalar.activation(V[:, b:b + 2, 0:D],
                                                         vps[:], AF.Copy)
                                else:
                                    for bi in range(2):
                                        nc.vector.tensor_tensor(
                                            out=V[:, b + bi, 0:D],
                                            in0=vps[:, bi, :],
                                            in1=rvb[:], op=ALU.add)
                        ncc_g = max(1, (G8 * P) // CC)
                        for cci in range(ncc_g):
                          cc = (g // G8) * ncc_g + cci
                          for dst, wsb, rbias in (
                                ((KT, wks, rks), (QT, wqs, rqs))
                                if "qk" not in ablate else ()):
                            for k2 in range(KD):
                                qps = paps3.tile([P, CC], F32, tag="qk")
                                hq = min(512, CC)
                                for h in range(0, CC, hq):
                                    nc.tensor.matmul(
                                        qps[:, h:h + hq],
                                        wsb[:, :, k2 * P:(k2 + 1) * P],
                                        xnT[:, :, cc * CC + h:
                                            cc * CC + h + hq],
                                        start=True, stop=True,
                                        perf_mode=mybir.MatmulPerfMode.DoubleRow)
                                use_act = (qkq_eng == "act" or
                                           (qkq_eng == "mix" and qki % 2 == 0))
                                qki += 1
                                if use_act:
                                    nc.scalar.activation(
                                        dst[:, k2, cc * CC:(cc + 1) * CC],
                                        qps[:], AF.Identity,
                                        bias=rbias[:, k2:k2 + 1], scale=1.0)
                                else:
                                    nc.vector.tensor_scalar_add(
                                        out=dst[:, k2, cc * CC:(cc + 1) * CC],
                                        in0=qps[:], scalar1=rbias[:, k2:k2 + 1])

                # ---- attention ----
                conv_i = 0
                if skip_attn:
                    nc.vector.tensor_reduce(
                        out=mv2[:, :, 0], in_=xsb[:, :, 0:2],
                        axis=mybir.AxisListType.X, op=ALU.add)
                    nc.vector.tensor_reduce(
                        out=mv2[:, :, 1], in_=xsb[:, :, 0:2],
                        axis=mybir.AxisListType.X, op=ALU.add)
                    nc.scalar.activation(mv2[:, :, 1], mv2[:, :, 1],
                                         AF.Sqrt, bias=epst[:], scale=1.0)
                    nc.vector.reciprocal(mv2[:, :, 1], mv2[:, :, 1])
                elif True:
                    with tc.tile_pool(name="at_pool", bufs=at_bufs) as atp, \
                       tc.tile_pool(name="attn_tmp", bufs=3) as att, \
                       tc.tile_pool(name="s_ps", bufs=sps_bufs, space="PSUM") as sps, \
                       tc.tile_pool(name="o_ps", bufs=ops_bufs, space="PSUM") as ops_:
                      if attn_loop:
                        QT_r = QT[:].rearrange("p k (g w) -> p g k w", g=NIG)
                        xsb_r = xsb[:].rearrange("p (g i) d -> p g i d", g=NIG)
                        mv2_r = mv2[:].rearrange("p (g i) s -> p g i s", g=NIG)
                        AT = atp.tile([P, JC, IG], F8, tag="AT")
                        QW = atp.tile([P, KD, IG], F8, tag="QW")
                        conv_i = 0
                        with tc.For_i(0, NIG, 1) as igv:
                          nc.vector.tensor_copy(QW[:, :, 0:512],
                                                QT_r[:, igv, :, 0:512])
                          nc.scalar.activation(QW[:, :, 512:IG],
                                               QT_r[:, igv, :, 512:IG],
                                               AF.Copy)
                          for j in range(JC):
                              sp = sps.tile([P, IG], F32, tag="s")
                              hs = min(512, IG)
                              for h in range(0, IG, hs):
                                  nc.tensor.matmul(
                                      sp[:, h:h + hs],
                                      KT[:, :, j * P:(j + 1) * P],
                                      QW[:, :, h:h + hs],
                                      start=True, stop=True,
                                      perf_mode=mybir.MatmulPerfMode.DoubleRow)
                              eng = conv_engines[conv_i % len(conv_engines)]
                              conv_i += 1
                              if eng == "act":
                                  nc.scalar.activation(
                                      AT[:, j, :], sp[:], AF.Exp, scale=SCALE_S)
                              elif eng == "pool":
                                  nc.gpsimd.tensor_scalar(
                                      out=AT[:, j, :].bitcast(U8), in0=sp[:],
                                      scalar1=float(A8), scalar2=float(B8),
                                      op0=ALU.mult, op1=ALU.add)
                              else:
                                  nc.vector.tensor_scalar(
                                      out=AT[:, j, :].bitcast(U8), in0=sp[:],
                                      scalar1=float(A8), scalar2=float(B8),
                                      op0=ALU.mult, op1=ALU.add)
                          for ib in range(IB):
                              op_ = ops_.tile([P, D + 2], F32, tag="o")
                              for jp in range(JC // 2):
                                  nc.tensor.matmul(
                                      op_[:],
                                      AT[:, 2 * jp:2 * jp + 2,
                                         ib * P:(ib + 1) * P],
                                      V[:, 2 * jp:2 * jp + 2, 0:D + 2],
                                      start=(jp == 0),
                                      stop=(jp == JC // 2 - 1),
                                      perf_mode=mybir.MatmulPerfMode.DoubleRow)
                              rec = att.tile([P, 1], F32, tag="rec")
                              nc.vector.reciprocal(rec[:], op_[:, D:D + 1])
                              nc.vector.scalar_tensor_tensor(
                                  out=xsb_r[:, igv, ib, :], in0=op_[:, 0:D],
                                  scalar=rec[:], in1=xsb_r[:, igv, ib, :],
                                  op0=ALU.mult, op1=ALU.add)
                              stats2 = att.tile([P, 6], F32, tag="st2")
                              nc.vector.bn_stats(stats2[:],
                                                 xsb_r[:, igv, ib, :])
                              nc.vector.bn_aggr(mv2_r[:, igv, ib, :],
                                                stats2[:])
                      else:
                        AT = None
                        for ig in range(NIG):
                            if "sexp" not in ablate and (AT is None or
                                                         at_bufs > 1):
                                AT = atp.tile([P, JC, IG], F8, tag="AT")
                            jstep = 2 if conv_pair else 1
                            for j in range(0, JC, jstep):
                                if "smm" not in ablate:
                                    sp = sps.tile([P, jstep, IG], F32, tag="s")
                                hs = min(512, IG)
                                for jj, h in (
                                        [] if "smm" in ablate else
                                        [(a, b) for a in range(jstep)
                                         for b in range(0, IG, hs)]):
                                    nc.tensor.matmul(
                                        sp[:, jj, h:h + hs],
                                        KT[:, :, (j + jj) * P:
                                           (j + jj + 1) * P],
                                        QT[:, :, ig * IG + h:ig * IG + h + hs],
                                        start=True, stop=True,
                                        perf_mode=mybir.MatmulPerfMode.DoubleRow)
                                eng = conv_engines[conv_i % len(conv_engines)]
                                conv_i += 1
                                spr = (sp[:] if "smm" not in ablate else
                                       xsb[:, 0:jstep * IG // D, :])
                                if "sexp" in ablate:
                                    pass
                                elif eng == "act":
                                    nc.scalar.activation(
                                        AT[:, j:j + jstep, :], spr, AF.Exp,
                                        scale=SCALE_S)
                                elif eng == "pool":
                                    nc.gpsimd.tensor_scalar(
                                        out=AT[:, j:j + jstep, :].bitcast(U8),
                                        in0=spr,
                                        scalar1=float(A8), scalar2=float(B8),
                                        op0=ALU.mult, op1=ALU.add)
                                else:
                                    nc.vector.tensor_scalar(
                                        out=AT[:, j:j + jstep, :].bitcast(U8),
                                        in0=spr,
                                        scalar1=float(A8), scalar2=float(B8),
                                        op0=ALU.mult, op1=ALU.add)
                            for ib in range(IB):
                                bb = ig * IB + ib
                                if not ("avmm" in ablate and
                                        "epi" in ablate):
                                    op_ = ops_.tile([P, D + 2], F32, tag="o")
                                for jp in ([] if "avmm" in ablate
                                           else range(JC // 2)):
                                    nc.tensor.matmul(
                                        op_[:],
                                        AT[:, 2 * jp:2 * jp + 2,
                                           ib * P:(ib + 1) * P]
                                        if "sexp" not in ablate else
                                        V[:, 2 * jp:2 * jp + 2, 0:P],
                                        V[:, 2 * jp:2 * jp + 2, 0:D + 2],
                                        start=(jp == 0),
                                        stop=(jp == JC // 2 - 1),
                                        perf_mode=mybir.MatmulPerfMode.DoubleRow)
                                if epi_mode == "div":
                                    nc.vector.scalar_tensor_tensor(
                                        out=xsb[:, bb, :], in0=op_[:, 0:D],
                                        scalar=op_[:, D:D + 1],
                                        in1=xsb[:, bb, :],
                                        op0=ALU.divide, op1=ALU.add)
                                    stats2d = att.tile([P, 6], F32, tag="st2")
                                    nc.vector.bn_stats(stats2d[:],
                                                       xsb[:, bb, :])
                                    nc.vector.bn_aggr(mv2[:, bb, :],
                                                      stats2d[:])
                                    continue
                                if "epi" in ablate:
                                    continue
                                rec = att.tile([P, 1], F32, tag="rec")
                                nc.vector.reciprocal(rec[:], op_[:, D:D + 1])
                                if epi_mode == "act":
                                    osb = att.tile([P, D], F32, tag="osb")
                                    nc.scalar.activation(osb[:], op_[:, 0:D],
                                                         AF.Copy, bias=0.0,
                                                         scale=rec[:])
                                    nc.gpsimd.tensor_add(out=xsb[:, bb, :],
                                                         in0=xsb[:, bb, :],
                                                         in1=osb[:])
                                else:
                                    nc.vector.scalar_tensor_tensor(
                                        out=xsb[:, bb, :], in0=op_[:, 0:D],
                                        scalar=rec[:], in1=xsb[:, bb, :],
                                        op0=ALU.mult, op1=ALU.add)
                            if not ln2_late and "epi" not in ablate:
                                ln_group_stats(mv2, ig * IB, IB, att, att)

            # ---- MLP ----
            if skip_mlp:
                out_r0 = out_d.ap().rearrange("(p tt) d -> p tt d", p=P)
                nc.sync.dma_start(out_r0[:, :, :], xsb[:])
            elif True:
                with tc.tile_pool(name="mlp_db", bufs=2) as mdb, \
                   tc.tile_pool(name="mlp_tmp", bufs=3) as mt, \
                   tc.tile_pool(name="m_tr", bufs=2, space="PSUM") as mps, \
                   tc.tile_pool(name="m_h", bufs=hps_bufs, space="PSUM") as hps, \
                   tc.tile_pool(name="m_y", bufs=2, space="PSUM") as yps:
                  out_r = out_d.ap().rearrange("(p tt) d -> p tt d", p=P)
                  if ln2_late and not skip_attn:
                      for _g in range(0, NB, G8):
                          for _bs in range(_g, _g + G8):
                              _st = mt.tile([P, 6], F32, tag="lst")
                              nc.vector.bn_stats(_st[:], xsb[:, _bs, :])
                              nc.vector.bn_aggr(mv2[:, _bs, :], _st[:])
                  if (attn_loop or defer_rstd or ln2_late) and not skip_attn:
                      if rsqrt:
                          nc.scalar.activation(mv2[:, :, 1], mv2[:, :, 1],
                                               AF.Rsqrt, bias=epst[:],
                                               scale=1.0)
                      else:
                          nc.scalar.activation(mv2[:, :, 1], mv2[:, :, 1],
                                               AF.Sqrt, bias=epst[:],
                                               scale=1.0)
                          nc.vector.reciprocal(mv2[:, :, 1], mv2[:, :, 1])
                  for tg in range(NTG):
                      xn2T = xn2T_all[:, tg % 2]
                      if ln_batch != "off" and "lnap2" not in ablate:
                          xt8m = mt.tile([P, TB, D], BF16, tag="xn2g")
                          ln_apply_group(xt8m[:], tg * TB, TB, mv2, mt)
                      for bp in range(0, TB, 2):
                          bb = tg * TB + bp
                          if ln_batch != "off" and "lnap2" not in ablate:
                              xt = xt8m[:, bp:bp + 2, :]
                          elif "lnap2" not in ablate:
                              xt = mt.tile([P, 2, D], BF16, tag="xn2")
                              ln_apply(xt[:, 0, :], xsb[:, bb, :], mv2, bb)
                              ln_apply(xt[:, 1, :], xsb[:, bb + 1, :], mv2,
                                       bb + 1)
                          if "mtr" in ablate:
                              continue
                          if tr_pack:
                              tpsF = mps.tile([P, 2, P], F32, tag="trf2")
                              for bi in range(2):
                                  nc.tensor.transpose(
                                      tpsF[:, bi, :],
                                      xt[:, bi, :].bitcast(F32), identf[:])
                              nc.vector.tensor_copy(
                                  xn2T[:, :, bp * P:(bp + 2) * P].rearrange(
                                      "p j (b q) -> p b j q", b=2),
                                  tpsF[:].bitcast(BF16).rearrange(
                                      "p b (t j) -> p b j t", j=2))
                              continue
                          if dma_tr:
                              for bi in range(2):
                                  for k in range(KD):
                                      nc.sync.dma_start_transpose(
                                          xn2T[:, k,
                                               (bp + bi) * P:(bp + bi + 1) * P],
                                          xt[:, bi, k * P:(k + 1) * P])
                              continue
                          tps = mps.tile([P, 2, KD, P], BF16, tag="tr2")
                          for bi in range(2):
                              for k in range(KD):
                                  nc.tensor.transpose(
                                      tps[:, bi, k, :],
                                      xt[:, bi, k * P:(k + 1) * P]
                                      if "lnap2" not in ablate else identb[:],
                                      identb[:])
                          nc.vector.tensor_copy(
                              xn2T[:, :, bp * P:(bp + 2) * P]
                              .rearrange("p k (b q) -> p b k q", b=2),
                              tps[:])
                      if "silu" not in ablate:
                          hT = mdb.tile([P, MO, TG], BF16, tag="hT")
                      for mo in range(MO):
                          if "hmm" not in ablate:
                              hp = hps.tile([P, TG], F32, tag="h")
                          hm = min(512, TG)
                          for h in ([] if "hmm" in ablate
                                    else range(0, TG, hm)):
                              for k in range(KD):
                                  nc.tensor.matmul(
                                      hp[:, h:h + hm],
                                      w1s[:, k, mo * P:(mo + 1) * P],
                                      xn2T[:, k, h:h + hm]
                                      if "mtr" not in ablate else
                                      w1s[:, k, h:h + hm],
                                      start=(k == 0), stop=(k == KD - 1))
                          if "silu" not in ablate:
                              nc.scalar.activation(hT[:, mo, :],
                                                   hp[:] if "hmm" not in ablate
                                                   else xn2T[:, 0, :],
                                                   silu_af,
                                                   bias=b1rs[:, mo:mo + 1],
                                                   scale=1.0)
                      otg = mdb.tile([P, TB, D], F32, tag="otg")
                      H2 = TB // 2
                      for bloc in range(TB):
                          bb = tg * TB + bloc
                          if "ymm" not in ablate:
                              yp = yps.tile([P, D], F32, tag="y")
                          for mo in ([] if "ymm" in ablate else range(MO)):
                              nc.tensor.matmul(yp[:],
                                               hT[:, mo, bloc * P:(bloc + 1) * P]
                                               if "silu" not in ablate else
                                               w1s[:, 0, bloc * P:(bloc + 1) * P],
                                               w2s[:, mo, :],
                                               start=(mo == 0), stop=(mo == MO - 1))
                          nc.vector.scalar_tensor_tensor(
                              out=otg[:, bloc, :],
                              in0=yp[:] if "ymm" not in ablate
                              else xsb[:, bb, :], scalar=1.0,
                              in1=xsb[:, bb, :], op0=ALU.mult, op1=ALU.add)
                          if not zb:
                              nc.gpsimd.tensor_add(out=otg[:, bloc, :],
                                                   in0=otg[:, bloc, :],
                                                   in1=b2b[:])
                          if bloc == H2 - 1 and not out_merge:
                              nc.sync.dma_start(
                                  out_r[:, tg * TB:tg * TB + H2, :],
                                  otg[:, 0:H2, :])
                      if out_merge:
                          nc.sync.dma_start(
                              out_r[:, tg * TB:(tg + 1) * TB, :], otg[:])
                      else:
                          nc.sync.dma_start(
                              out_r[:, tg * TB + H2:(tg + 1) * TB, :],
                              otg[:, H2:TB, :])

    nc.compile()
    return nc


def prepare_inputs(x, w_qkv, gamma1, beta1, gamma2, beta2, w1, b1, w2, b2,
                   tr_pack=False):
    """Host-side prep: slice w_qkv, fold gamma/beta into weights, transpose.
    Q/K/V weights are prescaled by WS=16 and cast to fp8e4m3; the 16x (and
    16x16 in the logits) is compensated by SCALE_S and the 16.0 ones column.
    """
    import ml_dtypes
    f8d = np.float64
    BF = ml_dtypes.bfloat16
    E4 = ml_dtypes.float8_e4m3
    x = np.asarray(x, np.float32)
    B = x.shape[0]
    T = x.shape[1] * x.shape[2]
    w_qkv = np.asarray(w_qkv, f8d)
    g1 = np.asarray(gamma1, f8d)
    be1 = np.asarray(beta1, f8d)
    g2 = np.asarray(gamma2, f8d)
    be2 = np.asarray(beta2, f8d)
    w1 = np.asarray(w1, f8d)
    w2 = np.asarray(w2, f8d)
    wq, wk, wv = w_qkv[0::3], w_qkv[1::3], w_qkv[2::3]
    f32c = lambda a: np.ascontiguousarray(a, np.float32)
    bfc = lambda a: np.ascontiguousarray(np.asarray(a, np.float32), BF)
    f8c = lambda a: np.ascontiguousarray(np.asarray(a, np.float32), E4)
    common = {
        "wqT8": f8c((wq * g1[None, :] * WS).T),
        "wkT8": f8c((wk * g1[None, :] * WS).T),
        "wvT8": f8c((wv * g1[None, :] * WS).T),
        "rq16": f32c(wq @ be1 * WS),
        "rk16": f32c(wk @ be1 * WS),
        "rv16": f32c(wv @ be1 * WS),
        "w1T": bfc((w1 * g2[None, :]).T),
        "b1r": f32c(np.asarray(b1, f8d) + w1 @ be2),
        "w2T": bfc(w2.T),
        "b2": f32c(b2),
        "onesv16_f8": np.array([WS] + [0.0] * 15, E4),
    }
    if tr_pack:
        idx = (2 * np.arange(128)[None, :] + np.arange(2)[:, None]).ravel()
        for kk in ("wqT8", "wkT8", "wvT8", "w1T"):
            common[kk] = np.ascontiguousarray(common[kk][idx])
    xf = x.reshape(B, T, x.shape[3])
    in_maps = [dict(common, x=np.ascontiguousarray(xf[c])) for c in range(B)]
    return in_maps


_CACHE = {}


BEST = dict(dma_tr=False, defer_rstd=True, hps_bufs=3, qkq_eng="act",
            cp_eng="mix", epi_mode="act", v_eng="dvecopy",
            conv_engines=("dve", "act"), ln_batch="dp", tr_pack=True,
            loop_unroll=2, tg_width=256, ig_width=512, conv_pair=True,
            at_bufs=3, cc_width=512, qk_bufs=4, out_merge=True)


def get_nc(zb):
    key = f"nc{int(zb)}"
    if key not in _CACHE:
        _CACHE[key] = build(zb=zb, **BEST)
    return _CACHE[key]


def kernel(x, w_qkv, gamma1, beta1, gamma2, beta2, w1, b1, w2, b2):
    x = np.asarray(x, np.float32)
    B, N, H, Dd = x.shape
    assert (B, N, H, Dd) == (8, 1024, 4, 256), x.shape
    in_maps = prepare_inputs(x, w_qkv, gamma1, beta1, gamma2, beta2,
                             w1, b1, w2, b2, tr_pack=BEST["tr_pack"])
    zb = (not np.any(in_maps[0]["rv16"])) and (not np.any(in_maps[0]["b2"]))
    nc = get_nc(zb)
    res = run_bass_kernel_spmd(nc, in_maps, core_ids=list(range(N_CORES)))
    out = np.stack([res.results[c]["out"] for c in range(B)], 0)
    return np.ascontiguousarray(out.reshape(B, N, H, Dd).astype(np.float32))

